# revision 1
# baseline (speedup 1.0000x reference)
"""Contrastive (NT-Xent) loss kernel for Trainium2, 8 NeuronCores SPMD.

Math (B=4096, D=256, T=0.5):
  z = l2norm(emb) rows; reps=[z_i; z_j] (8192 x 256); sim = reps @ reps.T
  denom_r = sum_{c != r} exp(sim[r,c]/T);  pos_m = z_i[m].z_j[m]
  loss = mean_r( ln(denom_r) - pos_r/T )

Per-core plan (core k owns reps rows: z_i rows [512k,512k+512) and z_j rows
[512k,512k+512) -> 8 m-tiles of 128):
  - load full emb_i/emb_j (replicated) + own row blocks (sharded)
  - rowwise sq-sums (DVE ttr), inv_norm = Exp(-0.5*Ln(s)) (ACT, same table
    set as the main exp/ln), normalize to fp16 (DVE tensor_scalar)
  - transpose to d-major zT [128d x cols] via DMA xbar (fp16, SBUF->SBUF)
  - for each 2048-col group g, m-tile: matmul fp16 -> PSUM fp32 [128,2048],
    ACT Exp(scale=2) in-place with accum_out -> per-row partial sums
  - rowsum -> ln(rowsum - e^2) (removes the diag term exactly enough),
    minus 4*sum(pos), -> per-partition partial [128,1] per core
Host: loss = sum(partials)/(2B).  (gather/unshard = sum of shards)
"""

import os
import numpy as np
from contextlib import ExitStack

import concourse.bass as bass
import concourse.tile as tile
from concourse import bacc, mybir
from concourse import bass_utils

B = 4096
D = 256
TEMP = 0.5
NCORES = 8
ROWS = 2 * B            # 8192 reps rows
PER = B // NCORES       # 512 rows of emb_i (and emb_j) per core
OWN = 2 * PER           # 1024 reps rows per core
P = 128
NG = 4                  # column groups
GCOLS = ROWS // NG      # 2048 columns per group
MT = OWN // P           # 8 m-tiles per core
F32 = mybir.dt.float32
DT = mybir.dt.float16   # matmul input dtype
INV_T = 1.0 / TEMP      # 2.0
DIAG = float(np.exp(np.float32(INV_T), dtype=np.float32))  # exp(2*||z||^2), ||z||~1

# "pe" = TensorE transpose + DVE evac (PSUM shared with matmul groups)
# "dma" = DMA xbar transpose SBUF->SBUF
TRANSPOSE_MODE = os.environ.get("CL_TRANSPOSE", "dma")


def _kernel_body(ctx: ExitStack, tc: tile.TileContext, out_ap, xi, xj, oa, ob):
    nc = tc.nc
    AF = mybir.ActivationFunctionType
    ALU = mybir.AluOpType

    x_pool = ctx.enter_context(tc.tile_pool(name="x", bufs=4))
    z_pool = ctx.enter_context(tc.tile_pool(name="z", bufs=4))
    zt_pool = ctx.enter_context(tc.tile_pool(name="zt", bufs=1))
    own_pool = ctx.enter_context(tc.tile_pool(name="own", bufs=1))
    st_pool = ctx.enter_context(tc.tile_pool(name="st", bufs=2))
    fin_pool = ctx.enter_context(tc.tile_pool(name="fin", bufs=1))
    ps_pool = ctx.enter_context(tc.tile_pool(name="ps", bufs=2, space="PSUM"))

    dummy = fin_pool.tile([P, 1], F32, tag="dummy")
    rowparts = fin_pool.tile([P, MT * NG], F32, tag="rowparts")
    negdiag = fin_pool.tile([P, 1], F32, tag="negdiag")
    nc.gpsimd.memset(negdiag[:], -DIAG)

    if TRANSPOSE_MODE == "pe":
        ident = fin_pool.tile([P, P], DT, tag="ident")
        from concourse.masks import make_identity
        make_identity(nc, ident[:])

    sq_pool = ctx.enter_context(tc.tile_pool(name="sq", bufs=2))

    def sqsum_x3(x3, nt, sqs_ap):
        # sqs_ap[p, t] = sum_d x3[p,t,d]^2  (one big mul + one 3D reduce)
        sq3 = sq_pool.tile([P, nt, D], F32, tag="sq3", name="sq3")
        nc.vector.tensor_mul(sq3[:], x3[:, 0:nt, :], x3[:, 0:nt, :])
        nc.vector.reduce_sum(out=sqs_ap, in_=sq3[:], axis=mybir.AxisListType.X)

    def inv_from_sqs(sqs_ap, inv_ap):
        # inv = s^-0.5 = Exp(-0.5*Ln(s)); Ln+Exp live in one ACT table set
        nc.scalar.activation(out=inv_ap, in_=sqs_ap, func=AF.Ln)
        nc.scalar.activation(out=inv_ap, in_=inv_ap, func=AF.Exp, scale=-0.5)

    def transpose_block(zt_tile, col0, z3, t, h):
        # zt_tile[:, col0:col0+128] = z3[:, t, h*128:(h+1)*128].T
        src = z3[:, t, h * P:(h + 1) * P]
        dst = zt_tile[:, col0:col0 + P]
        nc.sync.dma_start_transpose(out=dst, in_=src)

    def transpose_group(zt_lo, zt_hi, z3s, nt):
        # z3s: list of (z3, local_t) covering nt row-tiles in column order
        if TRANSPOSE_MODE == "pe":
            for h, zt_t in ((0, zt_lo), (1, zt_hi)):
                ps = ps_pool.tile([P, GCOLS], DT, tag="ps", name="ps_tr")
                for u, (z3, t) in enumerate(z3s):
                    nc.tensor.transpose(
                        ps[:, u * P:(u + 1) * P], z3[:, t, h * P:(h + 1) * P],
                        ident[:])
                nc.vector.tensor_copy(zt_t[:, 0:nt * P], ps[:, 0:nt * P])
        else:
            for h, zt_t in ((0, zt_lo), (1, zt_hi)):
                for u, (z3, t) in enumerate(z3s):
                    transpose_block(zt_t, u * P, z3, t, h)

    # ---------------- own-block prologue ----------------
    own_x = own_pool.tile([P, 2 * (PER // P), D], F32, tag="own_x")  # [128,8,256]
    nt_own = PER // P  # 4
    nc.sync.dma_start(own_x[:, 0:nt_own, :], oa.rearrange("(t p) d -> p t d", p=P))
    nc.sync.dma_start(own_x[:, nt_own:2 * nt_own, :], ob.rearrange("(t p) d -> p t d", p=P))

    sqs_own = own_pool.tile([P, 2 * nt_own], F32, tag="sqs_own")
    sqsum_x3(own_x, 2 * nt_own, sqs_own[:])
    inv_own = own_pool.tile([P, 2 * nt_own], F32, tag="inv_own")
    inv_from_sqs(sqs_own[:], inv_own[:])

    z_own = own_pool.tile([P, 2 * nt_own, D], DT, tag="z_own")
    for t in range(2 * nt_own):
        nc.vector.tensor_scalar_mul(
            out=z_own[:, t, :], in0=own_x[:, t, :], scalar1=inv_own[:, t:t + 1])

    zt_own = [own_pool.tile([P, OWN], DT, tag=f"zt_own{h}", name=f"zt_own{h}")
              for h in range(2)]
    transpose_group(zt_own[0], zt_own[1],
                    [(z_own, t) for t in range(2 * nt_own)], 2 * nt_own)

    # positives: pos_t = (x_a[t] . x_b[t]) * inv_a[t] * inv_b[t]
    pos_raw = own_pool.tile([P, nt_own], F32, tag="pos_raw")
    pr3 = sq_pool.tile([P, nt_own, D], F32, tag="sq3", name="pr3")
    nc.vector.tensor_mul(pr3[:], own_x[:, 0:nt_own, :], own_x[:, nt_own:2 * nt_own, :])
    nc.vector.reduce_sum(out=pos_raw[:], in_=pr3[:], axis=mybir.AxisListType.X)
    pos = own_pool.tile([P, nt_own], F32, tag="pos")
    nc.vector.tensor_mul(pos[:], pos_raw[:], inv_own[:, 0:nt_own])
    nc.vector.tensor_mul(pos[:], pos[:], inv_own[:, nt_own:2 * nt_own])

    # ---------------- full-rep group prologue ----------------
    zt = [[None, None] for _ in range(NG)]

    def prologue_group(g):
        src = xi if g < 2 else xj
        r0 = (g % 2) * GCOLS
        nt = GCOLS // P  # 16 row-tiles
        x3s = []
        for half in range(2):
            x3 = x_pool.tile([P, 8, D], F32, tag="x")
            rows = src[r0 + half * 1024: r0 + (half + 1) * 1024]
            nc.sync.dma_start(x3[:], rows.rearrange("(t p) d -> p t d", p=P))
            x3s.append(x3)
        sqs = st_pool.tile([P, nt], F32, tag="sqs")
        for half in range(2):
            sqsum_x3(x3s[half], 8, sqs[:, half * 8:(half + 1) * 8])
        inv = st_pool.tile([P, nt], F32, tag="inv")
        inv_from_sqs(sqs[:], inv[:])
        z3s = []
        for half in range(2):
            z3 = z_pool.tile([P, 8, D], DT, tag="z")
            for t in range(8):
                nc.vector.tensor_scalar_mul(
                    out=z3[:, t, :], in0=x3s[half][:, t, :],
                    scalar1=inv[:, half * 8 + t: half * 8 + t + 1])
            z3s.append(z3)
        zt[g][0] = zt_pool.tile([P, GCOLS], DT, tag=f"zt{g}_0", name=f"zt{g}_0")
        zt[g][1] = zt_pool.tile([P, GCOLS], DT, tag=f"zt{g}_1", name=f"zt{g}_1")
        transpose_group(zt[g][0], zt[g][1],
                        [(z3s[t // 8], t % 8) for t in range(nt)], nt)

    def main_unit(g, m):
        ps = ps_pool.tile([P, GCOLS], F32, tag="ps")
        nsub = GCOLS // 512
        for ns in range(nsub):
            nc.tensor.matmul(
                ps[:, ns * 512:(ns + 1) * 512],
                lhsT=zt_own[0][:, m * P:(m + 1) * P],
                rhs=zt[g][0][:, ns * 512:(ns + 1) * 512],
                start=True, stop=False)
        for ns in range(nsub):
            nc.tensor.matmul(
                ps[:, ns * 512:(ns + 1) * 512],
                lhsT=zt_own[1][:, m * P:(m + 1) * P],
                rhs=zt[g][1][:, ns * 512:(ns + 1) * 512],
                start=False, stop=True)
        nc.scalar.activation(
            out=ps[:], in_=ps[:], func=AF.Exp, scale=INV_T,
            accum_out=rowparts[:, m * NG + g: m * NG + g + 1])

    prologue_group(0)
    for g in range(NG):
        for m in range(MT // 2):
            main_unit(g, m)
        if g + 1 < NG:
            prologue_group(g + 1)
        for m in range(MT // 2, MT):
            main_unit(g, m)

    # ---------------- tail ----------------
    denom = fin_pool.tile([P, MT], F32, tag="denom")
    nc.vector.reduce_sum(
        out=denom[:], in_=rowparts[:].rearrange("p (m g) -> p m g", g=NG),
        axis=mybir.AxisListType.X)
    ln8 = fin_pool.tile([P, MT], F32, tag="ln8")
    nc.scalar.activation(out=ln8[:], in_=denom[:], func=AF.Ln, bias=negdiag[:])
    lnsum = fin_pool.tile([P, 1], F32, tag="lnsum")
    nc.vector.reduce_sum(out=lnsum[:], in_=ln8[:], axis=mybir.AxisListType.X)
    possum = fin_pool.tile([P, 1], F32, tag="possum")
    nc.vector.reduce_sum(out=possum[:], in_=pos[:], axis=mybir.AxisListType.X)
    partial = fin_pool.tile([P, 1], F32, tag="partial")
    # partial = lnsum - 2*INV_T*possum   (each pos appears for a z_i and a z_j row)
    nc.vector.tensor_scalar(
        out=partial[:], in0=possum[:], scalar1=-2.0 * INV_T, scalar2=lnsum[:],
        op0=ALU.mult, op1=ALU.add)
    nc.sync.dma_start(out_ap, partial[:])


_NC_CACHE = {}


def build_nc():
    key = TRANSPOSE_MODE
    if key in _NC_CACHE:
        return _NC_CACHE[key]
    nc = bacc.Bacc("TRN2", target_bir_lowering=False, debug=False,
                   enable_asserts=False, num_devices=NCORES)
    xi = nc.dram_tensor("xi", (B, D), F32, kind="ExternalInput").ap()
    xj = nc.dram_tensor("xj", (B, D), F32, kind="ExternalInput").ap()
    oa = nc.dram_tensor("oa", (PER, D), F32, kind="ExternalInput").ap()
    ob = nc.dram_tensor("ob", (PER, D), F32, kind="ExternalInput").ap()
    out = nc.dram_tensor("out", (P, 1), F32, kind="ExternalOutput").ap()
    with tile.TileContext(nc) as tc:
        with ExitStack() as ctx:
            _kernel_body(ctx, tc, out, xi, xj, oa, ob)
    nc.compile()
    _NC_CACHE[key] = nc
    return nc


def make_in_maps(emb_i, emb_j):
    emb_i = np.ascontiguousarray(np.asarray(emb_i, dtype=np.float32))
    emb_j = np.ascontiguousarray(np.asarray(emb_j, dtype=np.float32))
    maps = []
    for k in range(NCORES):
        maps.append({
            "xi": emb_i,
            "xj": emb_j,
            "oa": np.ascontiguousarray(emb_i[k * PER:(k + 1) * PER]),
            "ob": np.ascontiguousarray(emb_j[k * PER:(k + 1) * PER]),
        })
    return maps


def run(emb_i, emb_j, trace=False, **kw):
    nc = build_nc()
    res = bass_utils.run_bass_kernel_spmd(
        nc, make_in_maps(emb_i, emb_j), core_ids=list(range(NCORES)),
        trace=trace, **kw)
    partials = np.stack([r["out"] for r in res.results])  # [8,128,1]
    loss = np.float32(partials.astype(np.float64).sum() / ROWS)
    return loss, res


def kernel(emb_i, emb_j):
    loss, _ = run(emb_i, emb_j, trace=False)
    return np.asarray(loss, dtype=np.float32)



# revision 2
# speedup vs baseline: 4.0324x; 4.0324x over previous
"""Contrastive (NT-Xent) loss kernel for Trainium2, 8 NeuronCores SPMD.

v2: sharded fp16 inputs + on-device AllGather.
  Host->device traffic dominates wall-clock (axon tunnel ~45MB/s): the
  previous version shipped full emb_i/emb_j to all 8 cores (72MB/run).
  Now each core receives only its own 512-row chunk of emb_i and emb_j
  in fp16 (4MB total), normalizes locally, transposes to d-major, and
  AllGathers the fp16 d-major blocks on device (512KB -> 4MB). Column
  order after the gather is a per-core-block permutation of the
  reference order, which is irrelevant for row-wise denominator sums.

Math (B=4096, D=256, T=0.5):
  z = l2norm(emb) rows; reps=[z_i; z_j] (8192 x 256); sim = reps @ reps.T
  denom_r = sum_{c != r} exp(sim[r,c]/T);  pos_m = z_i[m].z_j[m]
  loss = mean_r( ln(denom_r) - pos_r/T )

Per-core plan (core k owns reps rows: z_i rows [512k,512k+512) and z_j
rows [512k,512k+512) -> 8 m-tiles of 128):
  - load own fp16 chunks; rowwise sq-sums (fp16*fp16 exact in f32),
    inv_norm = Exp(-0.5*Ln(s)) (ACT), normalize to fp16 (DVE)
  - DMA-xbar transpose own block to d-major zT_own [128, 2, 1024]
  - DMA zT_own -> DRAM bounce, AllGather over 8 cores -> [8*256, 1024],
    DMA back to SBUF zt_all [128, 2, 8192]
  - for each 2048-col group g, m-tile: matmul fp16 -> PSUM fp32
    [128,2048], ACT Exp(scale=2) in-place with accum_out -> per-row
    partial sums
  - rowsum -> ln(rowsum - e^2) (removes the diag term) minus 4*sum(pos)
    -> per-partition partial [128,1] per core
Host: loss = sum(partials)/(2B).  (gather/unshard = sum of shards)
"""

import numpy as np
from contextlib import ExitStack

import concourse.bass as bass
import concourse.tile as tile
from concourse import bacc, mybir
from concourse import bass_utils

B = 4096
D = 256
TEMP = 0.5
NCORES = 8
ROWS = 2 * B            # 8192 reps rows
PER = B // NCORES       # 512 rows of emb_i (and emb_j) per core
OWN = 2 * PER           # 1024 reps rows per core
P = 128
NG = 4                  # column groups
GCOLS = ROWS // NG      # 2048 columns per group
MT = OWN // P           # 8 m-tiles per core
NT = PER // P           # 4 row-tiles per input chunk
F32 = mybir.dt.float32
F16 = mybir.dt.float16
INV_T = 1.0 / TEMP      # 2.0
DIAG = float(np.exp(np.float32(INV_T), dtype=np.float32))  # exp(2*||z||^2), ||z||~1


def _kernel_body(ctx: ExitStack, tc: tile.TileContext, out_ap, oa, ob):
    nc = tc.nc
    AF = mybir.ActivationFunctionType
    ALU = mybir.AluOpType

    own_pool = ctx.enter_context(tc.tile_pool(name="own", bufs=1))
    fin_pool = ctx.enter_context(tc.tile_pool(name="fin", bufs=1))
    ps_pool = ctx.enter_context(tc.tile_pool(name="ps", bufs=2, space="PSUM"))
    dram = ctx.enter_context(tc.tile_pool(name="dram", bufs=1, space="DRAM"))

    rowparts = fin_pool.tile([P, MT * NG], F32, tag="rowparts")
    negdiag = fin_pool.tile([P, 1], F32, tag="negdiag")
    nc.gpsimd.memset(negdiag[:], -DIAG)

    # ---------------- own-block prologue ----------------
    own_x = own_pool.tile([P, 2 * NT, D], F16, tag="own_x")  # [128,8,256]
    nc.sync.dma_start(own_x[:, 0:NT, :], oa.rearrange("(t p) d -> p t d", p=P))
    nc.sync.dma_start(own_x[:, NT:2 * NT, :], ob.rearrange("(t p) d -> p t d", p=P))

    sq3 = own_pool.tile([P, 2 * NT, D], F32, tag="sq3")
    nc.vector.tensor_mul(sq3[:], own_x[:], own_x[:])
    sqs = own_pool.tile([P, 2 * NT], F32, tag="sqs")
    nc.vector.reduce_sum(out=sqs[:], in_=sq3[:], axis=mybir.AxisListType.X)
    # inv = s^-0.5 = Exp(-0.5*Ln(s)); Ln+Exp live in one ACT table set
    inv = own_pool.tile([P, 2 * NT], F32, tag="inv")
    nc.scalar.activation(out=inv[:], in_=sqs[:], func=AF.Ln)
    nc.scalar.activation(out=inv[:], in_=inv[:], func=AF.Exp, scale=-0.5)

    z_own = own_pool.tile([P, 2 * NT, D], F16, tag="z_own")
    for t in range(2 * NT):
        nc.vector.tensor_scalar_mul(
            out=z_own[:, t, :], in0=own_x[:, t, :], scalar1=inv[:, t:t + 1])

    # positives: pos_t = (x_a[t] . x_b[t]) * inv_a[t] * inv_b[t]
    pr3 = own_pool.tile([P, NT, D], F32, tag="pr3")
    nc.vector.tensor_mul(pr3[:], own_x[:, 0:NT, :], own_x[:, NT:2 * NT, :])
    pos = own_pool.tile([P, NT], F32, tag="pos")
    nc.vector.reduce_sum(out=pos[:], in_=pr3[:], axis=mybir.AxisListType.X)
    nc.vector.tensor_mul(pos[:], pos[:], inv[:, 0:NT])
    nc.vector.tensor_mul(pos[:], pos[:], inv[:, NT:2 * NT])

    # transpose own rows to d-major: zT_own[q, h, t*128+p] = z_own[p, t, h*128+q]
    zT_own = own_pool.tile([P, 2, OWN], F16, tag="zT_own")
    for h in range(2):
        for t in range(2 * NT):
            nc.sync.dma_start_transpose(
                out=zT_own[:, h, t * P:(t + 1) * P],
                in_=z_own[:, t, h * P:(h + 1) * P])

    # ---------------- all-gather d-major blocks ----------------
    cc_in = dram.tile([2 * P, OWN], F16, tag="cc_in")
    cc_out = dram.tile([NCORES * 2 * P, OWN], F16, tag="cc_out")
    nc.sync.dma_start(cc_in[:].rearrange("(h p) c -> p h c", p=P), zT_own[:])
    nc.gpsimd.collective_compute(
        "AllGather", mybir.AluOpType.bypass,
        replica_groups=[list(range(NCORES))],
        ins=[cc_in[:].opt()], outs=[cc_out[:].opt()])

    zt_all = own_pool.tile([P, 2, ROWS], F16, tag="zt_all")
    for g in range(NCORES):
        nc.sync.dma_start(
            zt_all[:, :, g * OWN:(g + 1) * OWN],
            cc_out[g * 2 * P:(g + 1) * 2 * P].rearrange("(h p) c -> p h c", p=P))

    # ---------------- main loop ----------------
    def main_unit(g, m):
        ps = ps_pool.tile([P, GCOLS], F32, tag="ps")
        nsub = GCOLS // 512
        for h in range(2):
            for ns in range(nsub):
                nc.tensor.matmul(
                    ps[:, ns * 512:(ns + 1) * 512],
                    lhsT=zT_own[:, h, m * P:(m + 1) * P],
                    rhs=zt_all[:, h, g * GCOLS + ns * 512:g * GCOLS + (ns + 1) * 512],
                    start=(h == 0), stop=(h == 1))
        nc.scalar.activation(
            out=ps[:], in_=ps[:], func=AF.Exp, scale=INV_T,
            accum_out=rowparts[:, m * NG + g: m * NG + g + 1])

    for g in range(NG):
        for m in range(MT):
            main_unit(g, m)

    # ---------------- tail ----------------
    denom = fin_pool.tile([P, MT], F32, tag="denom")
    nc.vector.reduce_sum(
        out=denom[:], in_=rowparts[:].rearrange("p (m g) -> p m g", g=NG),
        axis=mybir.AxisListType.X)
    ln8 = fin_pool.tile([P, MT], F32, tag="ln8")
    nc.scalar.activation(out=ln8[:], in_=denom[:], func=AF.Ln, bias=negdiag[:])
    lnsum = fin_pool.tile([P, 1], F32, tag="lnsum")
    nc.vector.reduce_sum(out=lnsum[:], in_=ln8[:], axis=mybir.AxisListType.X)
    possum = fin_pool.tile([P, 1], F32, tag="possum")
    nc.vector.reduce_sum(out=possum[:], in_=pos[:], axis=mybir.AxisListType.X)
    partial = fin_pool.tile([P, 1], F32, tag="partial")
    # partial = lnsum - 2*INV_T*possum   (each pos appears for a z_i and a z_j row)
    nc.vector.tensor_scalar(
        out=partial[:], in0=possum[:], scalar1=-2.0 * INV_T, scalar2=lnsum[:],
        op0=ALU.mult, op1=ALU.add)
    nc.sync.dma_start(out_ap, partial[:])


_NC_CACHE = {}


def build_nc():
    key = "v2"
    if key in _NC_CACHE:
        return _NC_CACHE[key]
    nc = bacc.Bacc("TRN2", target_bir_lowering=False, debug=False,
                   enable_asserts=False, num_devices=NCORES)
    oa = nc.dram_tensor("oa", (PER, D), F16, kind="ExternalInput").ap()
    ob = nc.dram_tensor("ob", (PER, D), F16, kind="ExternalInput").ap()
    out = nc.dram_tensor("out", (P, 1), F32, kind="ExternalOutput").ap()
    with tile.TileContext(nc) as tc:
        with ExitStack() as ctx:
            _kernel_body(ctx, tc, out, oa, ob)
    nc.compile()
    _NC_CACHE[key] = nc
    return nc


def make_in_maps(emb_i, emb_j):
    ei16 = np.asarray(emb_i, dtype=np.float32).astype(np.float16)
    ej16 = np.asarray(emb_j, dtype=np.float32).astype(np.float16)
    maps = []
    for k in range(NCORES):
        maps.append({
            "oa": np.ascontiguousarray(ei16[k * PER:(k + 1) * PER]),
            "ob": np.ascontiguousarray(ej16[k * PER:(k + 1) * PER]),
        })
    return maps


def run(emb_i, emb_j, trace=False, **kw):
    nc = build_nc()
    res = bass_utils.run_bass_kernel_spmd(
        nc, make_in_maps(emb_i, emb_j), core_ids=list(range(NCORES)),
        trace=trace, **kw)
    partials = np.stack([r["out"] for r in res.results])  # [8,128,1]
    loss = np.float32(partials.astype(np.float64).sum() / ROWS)
    return loss, res


def kernel(emb_i, emb_j):
    loss, _ = run(emb_i, emb_j, trace=False)
    return np.asarray(loss, dtype=np.float32)


# revision 3
# speedup vs baseline: 8.4266x; 2.0897x over previous
"""Contrastive (NT-Xent) loss kernel for Trainium2.

v5: single-core, hardware-looped, fp8 input.

The graded quantity here is wall-clock of a warm dispatch through
run_bass_kernel_spmd (the axon tunnel has no NTFF profiling), which
decomposes as: jit trace/lower (~40ms) + per-call BIR->NEFF compile
(scales with instruction count; ~105ms at ~300 instructions) + NEFF
wrap/ship (~50ms) + input transfer (~29MB/s tunnel) + device load/run.
On-device compute is ~1ms and irrelevant by comparison, so the design
minimizes program size and host->device bytes instead of FLOPs:

  - ONE NeuronCore does the whole job (adding cores only adds per-call
    collective comm setup, ~0.2s/call for an 8-core AllGather, and 8x
    the model loads; measured slower in every multi-core variant).
  - Input is a single fp8_e4m3 tensor [8192, 256] = 2MB ([emb_i; emb_j]
    rows, quantized on host: adds rel err ~7e-6 to the loss, tolerance
    is 2e-2). Cast fp8->fp16 on load via gpsimd DMA.
  - The 8192x8192 sim/exp/rowsum main loop runs as a hardware For_i
    over the 64 row-tiles (program: ~300 instructions instead of
    ~2400 unrolled; dynamic addressing confined to two DMAs per
    iteration: lhsT block load from a DRAM copy of zT, and the per-tile
    denominator store to DRAM).

Math (B=4096, D=256, T=0.5):
  z = l2norm(emb) rows; reps=[z_i; z_j] (8192 x 256); sim = reps @ reps.T
  denom_r = sum_{c != r} exp(sim[r,c]/T);  pos_m = z_i[m].z_j[m]
  loss = mean_r( ln(denom_r) - pos_r/T )

Device plan:
  - 4 groups of 2048 rows: cast-load fp8->fp16, rowwise sq-sums
    (fp16*fp16 exact in f32), inv_norm = Exp(-0.5*Ln(s)) (ACT),
    normalize to fp16 (DVE), DMA-xbar transpose into d-major
    zt_all [128, 2, 8192]; also mirror zT to DRAM (ztd) for the loop.
  - positives from the kept fp16 x tiles and inv norms.
  - For_i over m-tiles (stride 128): load lhsT [128,2,128] from ztd
    at dynamic column offset; 4 column groups x (8 matmuls fp16 ->
    PSUM f32 [128,2048] + ACT Exp(scale=2) with accum_out); reduce the
    4 partial sums, store [128,1] to denom_d[m*128:...] in DRAM.
  - tail: load denom_d as [128, 64], ln(denom - e^2) (removes diag),
    row-reduce, subtract 4*sum(pos) -> partial [128,1].
Host: loss = partial.sum() / 8192.
"""

import numpy as np
from contextlib import ExitStack

import ml_dtypes
import concourse.bass as bass
import concourse.tile as tile
from concourse import bacc, mybir
from concourse import bass_utils

B = 4096
D = 256
TEMP = 0.5
ROWS = 2 * B            # 8192 reps rows
P = 128
NGRP = 4                # normalization groups
GROWS = ROWS // NGRP    # 2048 rows per group
NT = GROWS // P         # 16 row-tiles per group
MT = ROWS // P          # 64 m-tiles
NG = 4                  # matmul column groups
GCOLS = ROWS // NG      # 2048 columns per group
F32 = mybir.dt.float32
F16 = mybir.dt.float16
F8 = mybir.dt.float8e4
INV_T = 1.0 / TEMP      # 2.0
DIAG = float(np.exp(np.float32(INV_T), dtype=np.float32))  # exp(2*||z||^2), ||z||~1


def _kernel_body(ctx: ExitStack, tc: tile.TileContext, out_ap, xall):
    nc = tc.nc
    AF = mybir.ActivationFunctionType
    ALU = mybir.AluOpType

    x_pool = ctx.enter_context(tc.tile_pool(name="x", bufs=2))
    sq_pool = ctx.enter_context(tc.tile_pool(name="sq", bufs=2))
    z_pool = ctx.enter_context(tc.tile_pool(name="z", bufs=2))
    st_pool = ctx.enter_context(tc.tile_pool(name="st", bufs=2))
    big_pool = ctx.enter_context(tc.tile_pool(name="big", bufs=1))
    fin_pool = ctx.enter_context(tc.tile_pool(name="fin", bufs=1))
    loop_pool = ctx.enter_context(tc.tile_pool(name="loop", bufs=1))
    ps_pool = ctx.enter_context(tc.tile_pool(name="ps", bufs=2, space="PSUM"))
    dram = ctx.enter_context(tc.tile_pool(name="dram", bufs=1, space="DRAM"))

    negdiag = fin_pool.tile([P, 1], F32, tag="negdiag")
    nc.gpsimd.memset(negdiag[:], -DIAG)
    zt_all = big_pool.tile([P, 2, ROWS], F16, tag="zt_all")
    inv_all = fin_pool.tile([P, MT], F32, tag="inv_all")
    x_keep = []

    # ---------------- prologue: normalize + transpose ----------------
    for g in range(NGRP):
        x3 = x_pool.tile([P, NT, D], F16, tag=f"x{g}", name=f"x{g}")
        rows = xall[g * GROWS:(g + 1) * GROWS]
        # gpsimd DMA casts fp8 -> fp16 during the load
        nc.gpsimd.dma_start(x3[:], rows.rearrange("(t p) d -> p t d", p=P))
        x_keep.append(x3)
        sq3 = sq_pool.tile([P, NT, D], F32, tag="sq3", name=f"sq3_{g}")
        nc.vector.tensor_mul(sq3[:], x3[:], x3[:])
        sqs = st_pool.tile([P, NT], F32, tag="sqs", name=f"sqs{g}")
        nc.vector.reduce_sum(out=sqs[:], in_=sq3[:], axis=mybir.AxisListType.X)
        # inv = s^-0.5 = Exp(-0.5*Ln(s)); Ln+Exp live in one ACT table set
        inv = inv_all[:, g * NT:(g + 1) * NT]
        nc.scalar.activation(out=inv, in_=sqs[:], func=AF.Ln)
        nc.scalar.activation(out=inv, in_=inv, func=AF.Exp, scale=-0.5)
        z3 = z_pool.tile([P, NT, D], F16, tag=f"z{g}", name=f"z{g}")
        for t in range(NT):
            nc.vector.tensor_scalar_mul(
                out=z3[:, t, :], in0=x3[:, t, :],
                scalar1=inv_all[:, g * NT + t:g * NT + t + 1])
        for h in range(2):
            for t in range(NT):
                nc.sync.dma_start_transpose(
                    out=zt_all[:, h, (g * NT + t) * P:(g * NT + t + 1) * P],
                    in_=z3[:, t, h * P:(h + 1) * P])

    # positives: emb_i row-tile t (groups 0,1) pairs with emb_j tile t (groups 2,3)
    pos = fin_pool.tile([P, MT // 2], F32, tag="pos")
    for half in range(2):
        pr3 = sq_pool.tile([P, NT, D], F32, tag="sq3", name=f"pr3_{half}")
        nc.vector.tensor_mul(pr3[:], x_keep[half][:], x_keep[2 + half][:])
        nc.vector.reduce_sum(
            out=pos[:, half * NT:(half + 1) * NT], in_=pr3[:],
            axis=mybir.AxisListType.X)
    nc.vector.tensor_mul(pos[:], pos[:], inv_all[:, 0:MT // 2])
    nc.vector.tensor_mul(pos[:], pos[:], inv_all[:, MT // 2:MT])

    # d-major mirror of z in DRAM for dynamic lhsT loads: ztd[(h p), c]
    ztd = dram.tile([2 * P, ROWS], F16, tag="ztd")
    nc.sync.dma_start(ztd[:].rearrange("(h p) c -> p h c", p=P), zt_all[:])
    denom_d = dram.tile([ROWS, 1], F32, tag="denom_d")

    # ---------------- hardware-looped main loop ----------------
    with tc.For_i(0, ROWS, P) as mP:
        lhsT = loop_pool.tile([P, 2, P], F16, tag="lhsT", name="lhsT")
        for h in range(2):
            nc.sync.dma_start(
                lhsT[:, h, :], ztd[h * P:(h + 1) * P, bass.ds(mP, P)])
        scratch = loop_pool.tile([P, NG], F32, tag="scratch", name="scratch")
        for g in range(NG):
            ps = ps_pool.tile([P, GCOLS], F32, tag="ps", name=f"ps{g}")
            for h in range(2):
                for ns in range(GCOLS // 512):
                    nc.tensor.matmul(
                        ps[:, ns * 512:(ns + 1) * 512],
                        lhsT=lhsT[:, h, :],
                        rhs=zt_all[:, h, g * GCOLS + ns * 512:g * GCOLS + (ns + 1) * 512],
                        start=(h == 0), stop=(h == 1))
            nc.scalar.activation(
                out=ps[:], in_=ps[:], func=AF.Exp, scale=INV_T,
                accum_out=scratch[:, g:g + 1])
        dsum = loop_pool.tile([P, 1], F32, tag="dsum", name="dsum")
        nc.vector.reduce_sum(out=dsum[:], in_=scratch[:], axis=mybir.AxisListType.X)
        nc.sync.dma_start(denom_d[bass.ds(mP, P)], dsum[:])

    # ---------------- tail ----------------
    denoms = fin_pool.tile([P, MT], F32, tag="denoms")
    nc.sync.dma_start(
        denoms[:], denom_d[:].rearrange("(m p) o -> p (m o)", p=P))
    ln64 = fin_pool.tile([P, MT], F32, tag="ln64")
    nc.scalar.activation(out=ln64[:], in_=denoms[:], func=AF.Ln, bias=negdiag[:])
    lnsum = fin_pool.tile([P, 1], F32, tag="lnsum")
    nc.vector.reduce_sum(out=lnsum[:], in_=ln64[:], axis=mybir.AxisListType.X)
    possum = fin_pool.tile([P, 1], F32, tag="possum")
    nc.vector.reduce_sum(out=possum[:], in_=pos[:], axis=mybir.AxisListType.X)
    partial = fin_pool.tile([P, 1], F32, tag="partial")
    # partial = lnsum - 2*INV_T*possum   (each pos appears for a z_i and a z_j row)
    nc.vector.tensor_scalar(
        out=partial[:], in0=possum[:], scalar1=-2.0 * INV_T, scalar2=lnsum[:],
        op0=ALU.mult, op1=ALU.add)
    nc.sync.dma_start(out_ap, partial[:])


_NC_CACHE = {}


def build_nc():
    key = "v5"
    if key in _NC_CACHE:
        return _NC_CACHE[key]
    nc = bacc.Bacc("TRN2", target_bir_lowering=False, debug=False,
                   enable_asserts=False, num_devices=1)
    xall = nc.dram_tensor("xall", (ROWS, D), F8, kind="ExternalInput").ap()
    out = nc.dram_tensor("out", (P, 1), F32, kind="ExternalOutput").ap()
    with tile.TileContext(nc) as tc:
        with ExitStack() as ctx:
            _kernel_body(ctx, tc, out, xall)
    nc.compile()
    _NC_CACHE[key] = nc
    return nc


def make_in_maps(emb_i, emb_j):
    xall = np.concatenate([
        np.asarray(emb_i, dtype=np.float32),
        np.asarray(emb_j, dtype=np.float32)])
    return [{"xall": np.ascontiguousarray(xall.astype(ml_dtypes.float8_e4m3))}]


def run(emb_i, emb_j, trace=False, **kw):
    nc = build_nc()
    res = bass_utils.run_bass_kernel_spmd(
        nc, make_in_maps(emb_i, emb_j), core_ids=[0], trace=trace, **kw)
    partial = res.results[0]["out"]  # [128, 1]
    loss = np.float32(partial.astype(np.float64).sum() / ROWS)
    return loss, res


def kernel(emb_i, emb_j):
    loss, _ = run(emb_i, emb_j, trace=False)
    return np.asarray(loss, dtype=np.float32)


# revision 4
# speedup vs baseline: 9.6368x; 1.1436x over previous
"""Contrastive (NT-Xent) loss kernel for Trainium2.

v5: single-core, hardware-looped, fp8 input.

The graded quantity here is wall-clock of a warm dispatch through
run_bass_kernel_spmd (the axon tunnel has no NTFF profiling), which
decomposes as: jit trace/lower (~40ms) + per-call BIR->NEFF compile
(scales with instruction count; ~105ms at ~300 instructions) + NEFF
wrap/ship (~50ms) + input transfer (~29MB/s tunnel) + device load/run.
On-device compute is ~1ms and irrelevant by comparison, so the design
minimizes program size and host->device bytes instead of FLOPs:

  - ONE NeuronCore does the whole job (adding cores only adds per-call
    collective comm setup, ~0.2s/call for an 8-core AllGather, and 8x
    the model loads; measured slower in every multi-core variant).
  - Input is a single fp8_e4m3 tensor [8192, 256] = 2MB ([emb_i; emb_j]
    rows, quantized on host: adds rel err ~7e-6 to the loss, tolerance
    is 2e-2). Cast fp8->fp16 on load via gpsimd DMA.
  - The 8192x8192 sim/exp/rowsum main loop runs as a hardware For_i
    over the 64 row-tiles (program: ~300 instructions instead of
    ~2400 unrolled; dynamic addressing confined to two DMAs per
    iteration: lhsT block load from a DRAM copy of zT, and the per-tile
    denominator store to DRAM).

Math (B=4096, D=256, T=0.5):
  z = l2norm(emb) rows; reps=[z_i; z_j] (8192 x 256); sim = reps @ reps.T
  denom_r = sum_{c != r} exp(sim[r,c]/T);  pos_m = z_i[m].z_j[m]
  loss = mean_r( ln(denom_r) - pos_r/T )

Device plan:
  - 4 groups of 2048 rows: cast-load fp8->fp16, rowwise sq-sums
    (fp16*fp16 exact in f32), inv_norm = Exp(-0.5*Ln(s)) (ACT),
    normalize to fp16 (DVE), DMA-xbar transpose into d-major
    zt_all [128, 2, 8192]; also mirror zT to DRAM (ztd) for the loop.
  - positives from the kept fp16 x tiles and inv norms.
  - For_i over m-tiles (stride 128): load lhsT [128,2,128] from ztd
    at dynamic column offset; 4 column groups x (8 matmuls fp16 ->
    PSUM f32 [128,2048] + ACT Exp(scale=2) with accum_out); reduce the
    4 partial sums, store [128,1] to denom_d[m*128:...] in DRAM.
  - tail: load denom_d as [128, 64], ln(denom - e^2) (removes diag),
    row-reduce, subtract 4*sum(pos) -> partial [128,1].
Host: loss = partial.sum() / 8192.
"""

import numpy as np
from contextlib import ExitStack

import ml_dtypes
import concourse.bass as bass
import concourse.tile as tile
from concourse import bacc, mybir
from concourse import bass_utils

B = 4096
D = 256
TEMP = 0.5
ROWS = 2 * B            # 8192 reps rows
P = 128
NGRP = 4                # normalization groups
GROWS = ROWS // NGRP    # 2048 rows per group
NT = GROWS // P         # 16 row-tiles per group
MT = ROWS // P          # 64 m-tiles
NG = 4                  # matmul column groups
GCOLS = ROWS // NG      # 2048 columns per group
F32 = mybir.dt.float32
F16 = mybir.dt.float16
F8 = mybir.dt.float8e4
INV_T = 1.0 / TEMP      # 2.0
DIAG = float(np.exp(np.float32(INV_T), dtype=np.float32))  # exp(2*||z||^2), ||z||~1


def _kernel_body(ctx: ExitStack, tc: tile.TileContext, out_ap, xall):
    nc = tc.nc
    AF = mybir.ActivationFunctionType
    ALU = mybir.AluOpType

    x_pool = ctx.enter_context(tc.tile_pool(name="x", bufs=2))
    sq_pool = ctx.enter_context(tc.tile_pool(name="sq", bufs=2))
    z_pool = ctx.enter_context(tc.tile_pool(name="z", bufs=2))
    st_pool = ctx.enter_context(tc.tile_pool(name="st", bufs=2))
    big_pool = ctx.enter_context(tc.tile_pool(name="big", bufs=1))
    fin_pool = ctx.enter_context(tc.tile_pool(name="fin", bufs=1))
    loop_pool = ctx.enter_context(tc.tile_pool(name="loop", bufs=1))
    ps_pool = ctx.enter_context(tc.tile_pool(name="ps", bufs=2, space="PSUM"))
    dram = ctx.enter_context(tc.tile_pool(name="dram", bufs=1, space="DRAM"))

    negdiag = fin_pool.tile([P, 1], F32, tag="negdiag")
    nc.gpsimd.memset(negdiag[:], -DIAG)
    zt_all = big_pool.tile([P, 2, ROWS], F16, tag="zt_all")
    inv_all = fin_pool.tile([P, MT], F32, tag="inv_all")
    x_keep = []

    # ---------------- prologue: normalize + transpose ----------------
    for g in range(NGRP):
        x3 = x_pool.tile([P, NT, D], F16, tag=f"x{g}", name=f"x{g}")
        rows = xall[g * GROWS:(g + 1) * GROWS]
        # gpsimd DMA casts fp8 -> fp16 during the load
        nc.gpsimd.dma_start(x3[:], rows.rearrange("(t p) d -> p t d", p=P))
        x_keep.append(x3)
        sq3 = sq_pool.tile([P, NT, D], F32, tag="sq3", name=f"sq3_{g}")
        nc.vector.tensor_mul(sq3[:], x3[:], x3[:])
        sqs = st_pool.tile([P, NT], F32, tag="sqs", name=f"sqs{g}")
        nc.vector.reduce_sum(out=sqs[:], in_=sq3[:], axis=mybir.AxisListType.X)
        # inv = s^-0.5 = Exp(-0.5*Ln(s)); Ln+Exp live in one ACT table set
        inv = inv_all[:, g * NT:(g + 1) * NT]
        nc.scalar.activation(out=inv, in_=sqs[:], func=AF.Ln)
        nc.scalar.activation(out=inv, in_=inv, func=AF.Exp, scale=-0.5)
        z3 = z_pool.tile([P, NT, D], F16, tag=f"z{g}", name=f"z{g}")
        for t in range(NT):
            nc.vector.tensor_scalar_mul(
                out=z3[:, t, :], in0=x3[:, t, :],
                scalar1=inv_all[:, g * NT + t:g * NT + t + 1])
        for h in range(2):
            for t in range(NT):
                nc.sync.dma_start_transpose(
                    out=zt_all[:, h, (g * NT + t) * P:(g * NT + t + 1) * P],
                    in_=z3[:, t, h * P:(h + 1) * P])

    # positives: emb_i row-tile t (groups 0,1) pairs with emb_j tile t (groups 2,3)
    pos = fin_pool.tile([P, MT // 2], F32, tag="pos")
    for half in range(2):
        pr3 = sq_pool.tile([P, NT, D], F32, tag="sq3", name=f"pr3_{half}")
        nc.vector.tensor_mul(pr3[:], x_keep[half][:], x_keep[2 + half][:])
        nc.vector.reduce_sum(
            out=pos[:, half * NT:(half + 1) * NT], in_=pr3[:],
            axis=mybir.AxisListType.X)
    nc.vector.tensor_mul(pos[:], pos[:], inv_all[:, 0:MT // 2])
    nc.vector.tensor_mul(pos[:], pos[:], inv_all[:, MT // 2:MT])

    # d-major mirror of z in DRAM for dynamic lhsT loads: ztd[(h p), c]
    ztd = dram.tile([2 * P, ROWS], F16, tag="ztd")
    nc.sync.dma_start(ztd[:].rearrange("(h p) c -> p h c", p=P), zt_all[:])
    denom_d = dram.tile([ROWS, 1], F32, tag="denom_d")

    # ---------------- hardware-looped main loop ----------------
    with tc.For_i(0, ROWS, P) as mP:
        lhsT = loop_pool.tile([P, 2, P], F16, tag="lhsT", name="lhsT")
        for h in range(2):
            nc.sync.dma_start(
                lhsT[:, h, :], ztd[h * P:(h + 1) * P, bass.ds(mP, P)])
        scratch = loop_pool.tile([P, NG], F32, tag="scratch", name="scratch")
        for g in range(NG):
            ps = ps_pool.tile([P, GCOLS], F32, tag="ps", name=f"ps{g}")
            for h in range(2):
                for ns in range(GCOLS // 512):
                    nc.tensor.matmul(
                        ps[:, ns * 512:(ns + 1) * 512],
                        lhsT=lhsT[:, h, :],
                        rhs=zt_all[:, h, g * GCOLS + ns * 512:g * GCOLS + (ns + 1) * 512],
                        start=(h == 0), stop=(h == 1))
            nc.scalar.activation(
                out=ps[:], in_=ps[:], func=AF.Exp, scale=INV_T,
                accum_out=scratch[:, g:g + 1])
        dsum = loop_pool.tile([P, 1], F32, tag="dsum", name="dsum")
        nc.vector.reduce_sum(out=dsum[:], in_=scratch[:], axis=mybir.AxisListType.X)
        nc.sync.dma_start(denom_d[bass.ds(mP, P)], dsum[:])

    # ---------------- tail ----------------
    denoms = fin_pool.tile([P, MT], F32, tag="denoms")
    nc.sync.dma_start(
        denoms[:], denom_d[:].rearrange("(m p) o -> p (m o)", p=P))
    ln64 = fin_pool.tile([P, MT], F32, tag="ln64")
    nc.scalar.activation(out=ln64[:], in_=denoms[:], func=AF.Ln, bias=negdiag[:])
    lnsum = fin_pool.tile([P, 1], F32, tag="lnsum")
    nc.vector.reduce_sum(out=lnsum[:], in_=ln64[:], axis=mybir.AxisListType.X)
    possum = fin_pool.tile([P, 1], F32, tag="possum")
    nc.vector.reduce_sum(out=possum[:], in_=pos[:], axis=mybir.AxisListType.X)
    partial = fin_pool.tile([P, 1], F32, tag="partial")
    # partial = lnsum - 2*INV_T*possum   (each pos appears for a z_i and a z_j row)
    nc.vector.tensor_scalar(
        out=partial[:], in0=possum[:], scalar1=-2.0 * INV_T, scalar2=lnsum[:],
        op0=ALU.mult, op1=ALU.add)
    nc.sync.dma_start(out_ap, partial[:])


_NC_CACHE = {}


def build_nc():
    key = "v5"
    if key in _NC_CACHE:
        return _NC_CACHE[key]
    nc = bacc.Bacc("TRN2", target_bir_lowering=False, debug=False,
                   enable_asserts=False, num_devices=1)
    xall = nc.dram_tensor("xall", (ROWS, D), F8, kind="ExternalInput").ap()
    out = nc.dram_tensor("out", (P, 1), F32, kind="ExternalOutput").ap()
    with tile.TileContext(nc) as tc:
        with ExitStack() as ctx:
            _kernel_body(ctx, tc, out, xall)
    nc.compile()
    _NC_CACHE[key] = nc
    return nc


def make_in_maps(emb_i, emb_j):
    xall = np.empty((ROWS, D), dtype=ml_dtypes.float8_e4m3)
    xall[:B] = np.asarray(emb_i)   # assignment casts f32 -> f8 in place
    xall[B:] = np.asarray(emb_j)
    return [{"xall": xall}]


def run(emb_i, emb_j, trace=False, **kw):
    nc = build_nc()
    res = bass_utils.run_bass_kernel_spmd(
        nc, make_in_maps(emb_i, emb_j), core_ids=[0], trace=trace, **kw)
    partial = res.results[0]["out"]  # [128, 1]
    loss = np.float32(partial.astype(np.float64).sum() / ROWS)
    return loss, res


def kernel(emb_i, emb_j):
    loss, _ = run(emb_i, emb_j, trace=False)
    return np.asarray(loss, dtype=np.float32)
